# revision 23
# baseline (speedup 1.0000x reference)
"""BiLevelRoutingAttention Trainium2 kernel (v2).

Strategy (8 NeuronCores, data-parallel over batch: 2 batches/core, 32 (b,t)
tiles per core):
  - Host: transpose x to feature-major bf16; compute the routing (region
    features from exact fp64 window sums, sim, top-4) in numpy and upload the
    additive window mask pre-expanded to key-token granularity
    (mwx[b,t,32*rg+n, jbq, ktok] = 0 if window selected else -1e9).
  - Device, per (b,t) tile, all layouts feature-major ("T-layout"):
      qT/kT = W^T x^T (bf16 matmuls, fp32 PSUM), bias add on DVE.
      V token-major (bias via ones-row matmul), PSUM->SBUF copy on DVE.
      ScoresT per (jbq, kb) chunk in a double-buffered 2-bank PSUM tile:
      4 head-row-tiled score matmuls + 4 mask matmuls (mask as weights,
      0/1 q-window expansion constant as moving operand), exp on ACT
      (scale folded) -> bf16 SBUF.
      Z via ones[128,32] matmuls (col-packed): the softmax denominator is
      materialized already broadcast across each head's 32 partitions in
      PSUM; 1/Z via DVE reciprocal_approx_fast (no DRAM bounce).
      PV col-packed, normalize on DVE, out-projection with bias folded in
      as a ones-row matmul, output DMA'd straight from PSUM.
"""

import sys

sys.path.insert(0, "/opt/trn_rl_repo")

import numpy as np
import ml_dtypes

import concourse.bass as bass
import concourse.bacc as bacc
import concourse.mybir as mybir
import concourse.tile as tile
from concourse.bass_utils import run_bass_kernel_spmd

BF16 = mybir.dt.bfloat16
F32 = mybir.dt.float32

NCORES = 8
B, T, S, C = 16, 16, 256, 256
NW, WIN, NH, D, TK = 8, 32, 8, 32, 4
BPC = B // NCORES  # batches per core
SCALE = float(D) ** -0.5
MASKVAL = -1e9

_CACHE = {}


def _build_nc(nt=T):
    nc = bacc.Bacc("TRN2", target_bir_lowering=False, debug=False)

    xt_d = nc.dram_tensor("xt", [BPC, nt, C, S], BF16, kind="ExternalInput")
    mwx_d = nc.dram_tensor("mwx", [BPC, nt, 128, 2, S], BF16, kind="ExternalInput")
    wqk_d = nc.dram_tensor("wqk_bf", [C, 2 * C], BF16, kind="ExternalInput")
    wv_d = nc.dram_tensor("wv_bf", [C, C], BF16, kind="ExternalInput")
    wp_d = nc.dram_tensor("wproj_bf", [C, C], BF16, kind="ExternalInput")
    bqk_d = nc.dram_tensor("bqk_cols", [128, 4], F32, kind="ExternalInput")
    bv_d = nc.dram_tensor("bv_row", [1, C], F32, kind="ExternalInput")
    bp_d = nc.dram_tensor("bproj_row", [1, C], F32, kind="ExternalInput")
    e8r_d = nc.dram_tensor("e8r", [128, S], BF16, kind="ExternalInput")
    out_d = nc.dram_tensor("out", [BPC, nt, 2, 128, C], BF16, kind="ExternalOutput")

    with tile.TileContext(nc) as tc:
        with (
            tc.tile_pool(name="wpool", bufs=1) as wp,
            tc.tile_pool(name="xpool", bufs=6) as xp,
            tc.tile_pool(name="mid", bufs=4) as mp,
            tc.tile_pool(name="exps", bufs=4) as ep,
            tc.tile_pool(name="b1", bufs=4, space="PSUM") as pb1,
            tc.tile_pool(name="sc", bufs=2, space="PSUM") as psc,
        ):
            # ---- weights / constants (loaded once) ----
            wqk_sb = wp.tile([128, 2, 2 * C], BF16)
            nc.sync.dma_start(out=wqk_sb, in_=wqk_d.ap().rearrange("(cc p) j -> p cc j", p=128))
            wv_sb = wp.tile([128, 2, C], BF16)
            nc.sync.dma_start(out=wv_sb, in_=wv_d.ap().rearrange("(cc p) j -> p cc j", p=128))
            wp_sb = wp.tile([128, 2, C], BF16)
            nc.sync.dma_start(out=wp_sb, in_=wp_d.ap().rearrange("(cc p) j -> p cc j", p=128))
            bqk_sb = wp.tile([128, 4], F32)
            nc.sync.dma_start(out=bqk_sb, in_=bqk_d.ap())
            e8r_sb = wp.tile([128, S], BF16)
            nc.sync.dma_start(out=e8r_sb, in_=e8r_d.ap())
            ones32_sb = wp.tile([128, 32], BF16)
            nc.vector.memset(ones32_sb, 1.0)
            # bias rows pre-broadcast to all 128 partitions (DMA supports
            # partition-step-0 source APs; DVE does not)
            bv_sb = wp.tile([128, C], F32)
            nc.sync.dma_start(out=bv_sb, in_=bv_d.ap().to_broadcast([128, C]))
            bp_sb = wp.tile([128, C], F32)
            nc.sync.dma_start(out=bp_sb, in_=bp_d.ap().to_broadcast([128, C]))

            for b in range(BPC):
                for t in range(nt):
                    _emit_tile(nc, tc, xp, mp, ep, pb1, psc,
                               xt_d, mwx_d, out_d, b, t,
                               wqk_sb, wv_sb, wp_sb, bqk_sb,
                               e8r_sb, ones32_sb, bv_sb, bp_sb)

    nc.compile()
    return nc


def _emit_tile(nc, tc, xp, mp, ep, pb1, psc, xt_d, mwx_d, out_d, b, t,
               wqk_sb, wv_sb, wp_sb, bqk_sb, e8r_sb, ones32_sb,
               bv_sb, bp_sb):
    AL = mybir.AluOpType

    # ---- load x^T and the pre-expanded routing mask ----
    xt_sb = xp.tile([128, 2, S], BF16, tag="xt")
    nc.sync.dma_start(out=xt_sb, in_=xt_d[b, t].rearrange("(cc p) s -> p cc s", p=128))
    mwx_sb = xp.tile([128, 2, S], BF16, tag="mwx")
    nc.sync.dma_start(out=mwx_sb, in_=mwx_d[b, t])

    # ---- qT / kT (feature-major); half h = (q_h, k_h) so the jbq=0 score
    # matmuls depend only on the first bias-add ----
    qk_sb = mp.tile([128, 4, S], BF16, tag="qk")  # slots (q0, k0, q1, k1)
    for half in range(2):
        qps = pb1.tile([128, 2, S], F32, tag="b1")
        for slot, jb in ((0, half), (1, 2 + half)):
            for cc in range(2):
                nc.tensor.matmul(qps[:, slot, :],
                                 lhsT=wqk_sb[:, cc, jb * 128:(jb + 1) * 128],
                                 rhs=xt_sb[:, cc, :],
                                 start=(slot == 0 and cc == 0),
                                 stop=(slot == 1 and cc == 1))
        nc.vector.tensor_tensor(
            out=qk_sb[:, 2 * half:2 * half + 2, :], in0=qps,
            in1=bqk_sb[:, 2 * half:2 * half + 2].unsqueeze(-1)
                .to_broadcast([128, 2, S]),
            op=AL.add)

    # ---- V (token-major), both blocks in one bank; bias folded into the
    # PSUM->SBUF copy on DVE ----
    v_sb = mp.tile([128, 2, C], BF16, tag="v")
    vps = pb1.tile([128, 2, C], F32, tag="b1")
    for sb_ in range(2):
        for cc in range(2):
            nc.tensor.matmul(vps[:, sb_, :],
                             lhsT=xt_sb[:, cc, sb_ * 128:(sb_ + 1) * 128],
                             rhs=wv_sb[:, cc, :],
                             start=(sb_ == 0 and cc == 0),
                             stop=(sb_ == 1 and cc == 1))
    nc.vector.tensor_tensor(out=v_sb, in0=vps,
                            in1=bv_sb[:].unsqueeze(1).to_broadcast([128, 2, C]),
                            op=AL.add)

    # ---- scoresT + mask + exp, per (jbq, rg-pair) chunk ----
    # chunk = [128, 2, 2S]: each row-group owns one PSUM bank; emission
    # alternates banks (score r0, score r1, mask r0, mask r1) so drains hide
    # behind the other bank's fill. psc bufs=2 keeps two chunks in flight
    # (4 concurrent row-group streams) and lets exp drain per 2-bank chunk,
    # shrinking the pipelining unit from the whole 4-bank score phase.
    expT = ep.tile([128, 2, 4, 2, S], BF16, tag="expT")  # (jbq, rg, kb, q)
    zp = pb1.tile([128, 2, S], F32, tag="b1")
    at = pb1.tile([128, 2, S], F32, tag="b1")
    chunks = [(jbq, rgp) for jbq in range(2) for rgp in range(2)]

    def _emit_scores(jbq, rgp):
        sc = psc.tile([128, 2, 2 * S], F32, tag="sc")
        for kb in range(2):
            for r in range(2):
                rg = 2 * rgp + r
                nc.tensor.matmul(
                    sc[:, r, kb * S:(kb + 1) * S],
                    lhsT=qk_sb[32 * rg:32 * rg + 32, 2 * jbq + 1,
                               kb * 128:(kb + 1) * 128],
                    rhs=qk_sb[32 * rg:32 * rg + 32, 2 * jbq, :],
                    start=(kb == 0), stop=False,
                    skip_group_check=True, tile_position=(32 * rg, 0))
            for r in range(2):
                rg = 2 * rgp + r
                nc.tensor.matmul(
                    sc[:, r, kb * S:(kb + 1) * S],
                    lhsT=mwx_sb[32 * rg:32 * rg + 8, jbq,
                                kb * 128:(kb + 1) * 128],
                    rhs=e8r_sb[32 * rg:32 * rg + 8, :],
                    start=False, stop=(kb == 1),
                    skip_group_check=True, tile_position=(32 * rg, 0))
        nc.scalar.activation(
            out=expT[:, jbq, 2 * rgp:2 * rgp + 2]
                .rearrange("p r k q -> p r (k q)"),
            in_=sc, func=mybir.ActivationFunctionType.Exp, scale=SCALE)

    def _emit_zpv(ci):
        jbq, rgp = chunks[ci]
        for kb in range(2):
            for r in range(2):  # both Z (zp bank), then both PV (at bank)
                rg = 2 * rgp + r
                nc.tensor.matmul(zp[32 * rg:32 * rg + 32, jbq, :],
                                 lhsT=ones32_sb,
                                 rhs=expT[:, jbq, rg, kb, :],
                                 start=(jbq == 0 and kb == 0),
                                 stop=(jbq == 1 and kb == 1),
                                 skip_group_check=True,
                                 tile_position=(0, 32 * rg))
            for r in range(2):
                rg = 2 * rgp + r
                hh = 4 * jbq + rg
                nc.tensor.matmul(at[32 * rg:32 * rg + 32, jbq, :],
                                 lhsT=v_sb[:, kb, 32 * hh:32 * hh + 32],
                                 rhs=expT[:, jbq, rg, kb, :],
                                 start=(jbq == 0 and kb == 0),
                                 stop=(jbq == 1 and kb == 1),
                                 skip_group_check=True,
                                 tile_position=(0, 32 * rg))

    _emit_scores(*chunks[0])
    for ci in range(len(chunks)):
        if ci + 1 < len(chunks):
            _emit_scores(*chunks[ci + 1])
        _emit_zpv(ci)

    # ---- normalize: rf = ~1/Z (already broadcast per head row-group) ----
    rf_sb = mp.tile([128, 2, S], F32, tag="rf")
    nc.vector.reciprocal_approx_fast(out=rf_sb, in_=zp)
    atn_sb = mp.tile([128, 2, S], BF16, tag="atn")
    nc.vector.tensor_tensor(out=atn_sb, in0=at, in1=rf_sb, op=AL.mult)

    # ---- out projection (both s-blocks in one bank) ----
    po = pb1.tile([128, 2, C], F32, tag="b1")
    for sb_ in range(2):
        for cc in range(2):
            nc.tensor.matmul(po[:, sb_, :],
                             lhsT=atn_sb[:, cc, sb_ * 128:(sb_ + 1) * 128],
                             rhs=wp_sb[:, cc, :],
                             start=(sb_ == 0 and cc == 0),
                             stop=(sb_ == 1 and cc == 1))
    out_sb = mp.tile([128, 2, C], BF16, tag="out")
    nc.vector.tensor_tensor(out=out_sb, in0=po,
                            in1=bp_sb[:].unsqueeze(1).to_broadcast([128, 2, C]),
                            op=AL.add)
    nc.sync.dma_start(out=out_d[b, t].rearrange("s p c -> p s c"),
                      in_=out_sb)


def _host_routing_mask(x4, w_qkv, b_qkv):
    """Top-4 window routing per (b, t, head, q-window), additive mask
    pre-expanded to key tokens in the device layout [B, T, 128, 2, S]."""
    bf16 = ml_dtypes.bfloat16
    # exact window sums -> region features (linearity of the projection)
    xsum = x4.reshape(B, T, NW, WIN, C).sum(3, dtype=np.float64)  # [B,T,NW,C]
    wq = w_qkv[:, :C].astype(np.float64)
    wk = w_qkv[:, C:2 * C].astype(np.float64)
    q_reg = xsum @ wq + WIN * b_qkv[:C].astype(np.float64)
    k_reg = xsum @ wk + WIN * b_qkv[C:2 * C].astype(np.float64)
    # [B,T,NW,NH,D]
    q_reg = q_reg.reshape(B, T, NW, NH, D)
    k_reg = k_reg.reshape(B, T, NW, NH, D)
    sim = np.einsum('btnhd,btmhd->bthnm', q_reg, k_reg)  # [B,T,h,qw,kw]
    # top-4 key windows per (b,t,h,qw)
    thr = np.partition(sim, NW - TK, axis=-1)[..., NW - TK:NW - TK + 1]
    sel = sim >= thr  # [B,T,h,qw,kw] (>=4 True; ties broken towards more)
    # guard: exact ties could select >4 windows; fall back to argsort there
    extra = sel.sum(-1) > TK
    if extra.any():
        order = np.argsort(-sim, axis=-1, kind='stable')[..., :TK]
        sel = np.zeros_like(sel)
        np.put_along_axis(sel, order, True, axis=-1)
    m8 = np.where(sel, np.float32(0.0), np.float32(MASKVAL))  # [B,T,h,qw,kw]
    m8 = np.repeat(m8, WIN, axis=-1)  # [B,T,h,qw,S] key-token level
    mwx = np.zeros((B, T, 128, 2, S), dtype=bf16)
    for jbq in range(2):
        for rg in range(4):
            h = 4 * jbq + rg
            mwx[:, :, 32 * rg:32 * rg + NW, jbq, :] = \
                m8[:, :, h].astype(bf16)
    return mwx


def _make_e8r():
    e = np.zeros((128, S), ml_dtypes.bfloat16)
    q = np.arange(S) // WIN  # query window of column q
    for rg in range(4):
        for n in range(NW):
            e[32 * rg + n, q == n] = 1.0
    return e


def _host_prep(x, w_qkv, b_qkv, w_proj, b_proj):
    bf16 = ml_dtypes.bfloat16
    x4 = x.reshape(B, T, S, C)
    xt = np.ascontiguousarray(x4.transpose(0, 1, 3, 2)).astype(bf16)
    mwx = _host_routing_mask(x4, w_qkv, b_qkv)

    shared = {
        "wqk_bf": np.ascontiguousarray(w_qkv[:, :2 * C]).astype(bf16),
        "wv_bf": np.ascontiguousarray(w_qkv[:, 2 * C:]).astype(bf16),
        "wproj_bf": w_proj.astype(bf16),
        "bqk_cols": np.ascontiguousarray(
            b_qkv[:2 * C].reshape(4, 128)[[0, 2, 1, 3]].T).astype(np.float32),
        "bv_row": b_qkv[2 * C:].reshape(1, C).astype(np.float32),
        "bproj_row": b_proj.reshape(1, C).astype(np.float32),
        "e8r": _make_e8r(),
    }
    in_maps = []
    for core in range(NCORES):
        b0 = core * BPC
        m = dict(shared)
        m["xt"] = np.ascontiguousarray(xt[b0:b0 + BPC])
        m["mwx"] = np.ascontiguousarray(mwx[b0:b0 + BPC])
        in_maps.append(m)
    return in_maps


def kernel(x, w_qkv, b_qkv, w_proj, b_proj, **_unused_scalars):
    x = np.asarray(x, dtype=np.float32)
    w_qkv = np.asarray(w_qkv, dtype=np.float32)
    b_qkv = np.asarray(b_qkv, dtype=np.float32)
    w_proj = np.asarray(w_proj, dtype=np.float32)
    b_proj = np.asarray(b_proj, dtype=np.float32)

    if "nc" not in _CACHE:
        _CACHE["nc"] = _build_nc()
    nc = _CACHE["nc"]

    in_maps = _host_prep(x, w_qkv, b_qkv, w_proj, b_proj)
    res = run_bass_kernel_spmd(nc, in_maps, core_ids=list(range(NCORES)))

    out = np.empty((B, T, 2, 128, C), np.float32)
    for core in range(NCORES):
        out[core * BPC:(core + 1) * BPC] = \
            np.asarray(res.results[core]["out"]).astype(np.float32)
    # [B, T, sb, p, C] -> [B, T*S, C]
    return out.reshape(B, T * S, C)
